# revision 6
# baseline (speedup 1.0000x reference)
"""Trainium2 Bass kernel for a YOLO-style detection loss.

Strategy (data-parallel over batch, per sharding hint):
  - Shard preds on batch dim: 4 images per core across 8 cores; partition
    targets by image index so each core's targets hit only its pred shard.
  - The loss only touches pred[b, 0, gy, gx] for each target, so each core
    performs an on-device indirect-DMA gather of its target rows (11 floats
    each) from its pred shard in HBM, then computes the elementwise
    softplus/L1 terms and reduces to three partial sums (box/obj/cls) with
    the gain/(1/n) constants folded into host-precomputed weights.
  - Host sums the 8 per-core partial triples (the all-reduce of scalar loss
    sums) and assembles (loss, lbox, lobj, lcls).
"""

import numpy as np

P = 128
NCLS = 6
NO = NCLS + 5
BS = 32
NA = 3
NCORES = 8
BPC = BS // NCORES  # images per core
LAYERS = ((160, 160), (80, 80), (40, 40))  # (ny, nx)
BOX_GAIN, CLS_GAIN, DFL_GAIN = 7.5, 0.5, 1.5

_BUILD_CACHE: dict = {}


def _emit_body(nc, pool, preds, idx_t, aux_t, out_d, K):
    """Emit one full loss-computation body (gather + compute + out DMA)."""
    from concourse import bass, mybir
    from concourse.bass_isa import ReduceOp

    S = 3 * K
    f32 = mybir.dt.float32
    add = mybir.AluOpType.add
    mult = mybir.AluOpType.mult

    # HW indirect DMA applies ONE offset per partition (walrus dynamic-AP
    # semantics; the multi-offset-per-partition form only exists in CoreSim),
    # so emit one gather instruction per slot column: each gathers 128 rows
    # of 11 floats, one row per partition.
    G = pool.tile([P, S * NO], f32)
    for s in range(S):
        nc.gpsimd.indirect_dma_start(
            out=G[:, s * NO : (s + 1) * NO],
            out_offset=None,
            in_=preds[s // K],
            in_offset=bass.IndirectOffsetOnAxis(ap=idx_t[:, s : s + 1], axis=1),
        )

    G3 = G[:].rearrange("p (s f) -> p s f", f=NO)
    T3 = aux_t[:, 0 : 4 * S].rearrange("p (s f) -> p s f", f=4)
    W23 = aux_t[:, 4 * S : 10 * S].rearrange("p (s f) -> p s f", f=6)
    wb = aux_t[:, 10 * S : 11 * S]
    wo = aux_t[:, 11 * S : 12 * S]
    wc = aux_t[:, 12 * S : 13 * S]

    # softplus(G) = ln(exp(G) + 1)
    E = pool.tile([P, S * NO], f32)
    SP = pool.tile([P, S * NO], f32)
    nc.scalar.activation(E[:], G[:], mybir.ActivationFunctionType.Exp)
    nc.scalar.activation(SP[:], E[:], mybir.ActivationFunctionType.Ln, bias=1.0)
    SP3 = SP[:].rearrange("p (s f) -> p s f", f=NO)

    # box: sum_s wb[s] * sum_{f<4} |G[s,f] - T[s,f]|
    D = pool.tile([P, S * 4], f32)
    D3 = D[:].rearrange("p (s f) -> p s f", f=4)
    nc.vector.tensor_sub(out=D3, in0=G3[:, :, 0:4], in1=T3)
    boxrow = pool.tile([P, S], f32)
    nc.vector.tensor_reduce(
        out=boxrow[:],
        in_=D3,
        axis=mybir.AxisListType.X,
        op=add,
        apply_absolute_value=True,
    )
    # masked (gain-folded) dot products; tensor_tensor_reduce hangs the HW
    # through this compile path, so use plain mul + reduce pairs instead.
    scratch = pool.tile([P, S], f32)
    scratch6 = pool.tile([P, S * 6], f32)
    part = pool.tile([P, 4], f32)  # box | obj | clsA(+) | clsB(-)
    nc.vector.tensor_mul(out=scratch[:], in0=boxrow[:], in1=wb)
    nc.vector.tensor_reduce(
        out=part[:, 0:1], in_=scratch[:], axis=mybir.AxisListType.X, op=add
    )

    # obj: softplus(-x) = softplus(x) - x
    objrow = pool.tile([P, S], f32)
    nc.vector.tensor_sub(out=objrow[:], in0=SP3[:, :, 4:5], in1=G3[:, :, 4:5])
    nc.vector.tensor_mul(out=objrow[:], in0=objrow[:], in1=wo)
    nc.vector.tensor_reduce(
        out=part[:, 1:2], in_=objrow[:], axis=mybir.AxisListType.X, op=add
    )

    # cls: sum_s wc[s]*sum_j softplus(G[s,5+j]) - sum_{s,j} G[s,5+j]*W2[s,j]
    sprow = pool.tile([P, S], f32)
    nc.vector.tensor_reduce(
        out=sprow[:], in_=SP3[:, :, 5:11], axis=mybir.AxisListType.X, op=add
    )
    nc.vector.tensor_mul(out=sprow[:], in0=sprow[:], in1=wc)
    nc.vector.tensor_reduce(
        out=part[:, 2:3], in_=sprow[:], axis=mybir.AxisListType.X, op=add
    )
    S63 = scratch6[:].rearrange("p (s f) -> p s f", f=6)
    nc.vector.tensor_mul(out=S63, in0=G3[:, :, 5:11], in1=W23)
    nc.vector.tensor_reduce(
        out=part[:, 3:4], in_=S63, axis=mybir.AxisListType.XY, op=add
    )

    final = pool.tile([P, 3], f32)
    nc.vector.tensor_copy(out=final[:, 0:2], in_=part[:, 0:2])
    nc.vector.tensor_sub(out=final[:, 2:3], in0=part[:, 2:3], in1=part[:, 3:4])

    red = pool.tile([P, 3], f32)
    nc.gpsimd.partition_all_reduce(red[:], final[:], P, ReduceOp.add)
    nc.sync.dma_start(out=out_d[:], in_=red[0:1, :])


def _build(K: int, repeat: int = 1):
    """Build + compile the per-core Bass program for capacity K*128 targets
    per core (per layer). repeat>1 unrolls the body for benchmarking only;
    the graded path uses repeat=1."""
    from concourse import bacc, mybir, tile

    S = 3 * K  # slots per partition across the 3 layers
    f32 = mybir.dt.float32

    nc = bacc.Bacc(
        "TRN2", target_bir_lowering=False, debug=False, enable_asserts=False
    )

    preds = []
    for l, (ny, nx) in enumerate(LAYERS):
        preds.append(
            nc.dram_tensor(
                f"pred{l}", [BPC * NA * ny, nx, NO], f32, kind="ExternalInput"
            ).ap()
        )
    idx_d = nc.dram_tensor("idx", [P, S], mybir.dt.int32, kind="ExternalInput").ap()
    # aux layout per partition (f32): T[S,4] | W2[S,6] | wb[S] | wo[S] | wc[S]
    aux_d = nc.dram_tensor("aux", [P, S * 13], f32, kind="ExternalInput").ap()
    out_d = nc.dram_tensor("out", [1, 3], f32, kind="ExternalOutput").ap()

    with tile.TileContext(nc) as tc:
        with tc.tile_pool(name="pool", bufs=1) as pool:
            idx_t = pool.tile([P, S], mybir.dt.int32)
            aux_t = pool.tile([P, S * 13], f32)
            nc.sync.dma_start(out=idx_t[:], in_=idx_d[:])
            nc.sync.dma_start(out=aux_t[:], in_=aux_d[:])
            for _rep in range(repeat):
                _emit_body(nc, pool, preds, idx_t, aux_t, out_d, K)

    nc.compile()
    return nc


def _prepare_in_maps(pred_full, targets):
    """Shard inputs and build per-core index/aux tensors. Returns (K, in_maps)."""
    n = targets.shape[0]
    inv_n = 1.0 / max(1, n)
    b = targets[:, 0].astype(np.int32)
    c = targets[:, 1].astype(np.int32)
    txywh = targets[:, 2:6].astype(np.float32)

    # per-layer grid cells + row index within a per-core shard
    rows_by_layer = []
    for ny, nx in LAYERS:
        gx = np.clip(
            np.floor(np.float32(nx) * txywh[:, 0]).astype(np.int32), 0, nx - 1
        )
        gy = np.clip(
            np.floor(np.float32(ny) * txywh[:, 1]).astype(np.int32), 0, ny - 1
        )
        rows_by_layer.append(((b % BPC) * NA * ny + gy) * nx + gx)

    core_of = b // BPC
    counts = np.bincount(core_of, minlength=NCORES)
    K = max(1, -(-int(counts.max()) // P))  # ceil(max_count/128)
    C = K * P
    S = 3 * K

    onehot = np.zeros((n, NCLS), dtype=np.float32)
    onehot[np.arange(n), np.clip(c, 0, NCLS - 1)] = 1.0

    in_maps = []
    for i in range(NCORES):
        sel = np.nonzero(core_of == i)[0]
        m = len(sel)

        def pad_pk(a):
            """pad per-target array [n,...] to [C, ...] then -> [P, K, ...]"""
            out = np.zeros((C,) + a.shape[1:], dtype=a.dtype)
            out[:m] = a[sel]
            return np.ascontiguousarray(
                out.reshape((K, P) + a.shape[1:]).swapaxes(0, 1)
            )

        idx = np.concatenate(
            [pad_pk(r.reshape(-1, 1))[:, :, 0] for r in rows_by_layer], axis=1
        ).astype(np.int32)  # [P, 3K]

        T_pk = pad_pk(txywh)  # [P, K, 4]
        W2_pk = pad_pk(onehot * np.float32(CLS_GAIN * inv_n))  # [P, K, 6]
        w_real = np.zeros((C,), dtype=np.float32)
        w_real[:m] = 1.0
        w_pk = np.ascontiguousarray(w_real.reshape(K, P).T)  # [P, K]

        T_s = np.concatenate([T_pk] * 3, axis=1).reshape(P, -1)
        W2_s = np.concatenate([W2_pk] * 3, axis=1).reshape(P, -1)
        w_s = np.concatenate([w_pk] * 3, axis=1)
        aux = np.concatenate(
            [
                T_s,
                W2_s,
                w_s * np.float32(BOX_GAIN * inv_n),
                w_s * np.float32(DFL_GAIN * inv_n),
                w_s * np.float32(CLS_GAIN * inv_n),
            ],
            axis=1,
        ).astype(np.float32)  # [P, 13*S]
        assert aux.shape == (P, 13 * S)

        in_map = {"idx": idx, "aux": np.ascontiguousarray(aux)}
        for l, (ny, nx) in enumerate(LAYERS):
            shard = pred_full[l][i * BPC : (i + 1) * BPC]
            in_map[f"pred{l}"] = np.ascontiguousarray(shard).reshape(
                BPC * NA * ny, nx, NO
            )
        in_maps.append(in_map)

    return K, in_maps


def _run(pred_full, targets, trace=False, **run_kwargs):
    from concourse import bass_utils

    K, in_maps = _prepare_in_maps(pred_full, targets)
    if K not in _BUILD_CACHE:
        _BUILD_CACHE[K] = _build(K)
    nc = _BUILD_CACHE[K]
    res = bass_utils.run_bass_kernel_spmd(
        nc, in_maps, core_ids=list(range(NCORES)), trace=trace, **run_kwargs
    )
    sums = np.zeros(3, dtype=np.float64)
    for r in res.results:
        sums += r["out"][0].astype(np.float64)
    lbox, lobj, lcls = sums.astype(np.float32)
    loss = np.float32(lbox + lobj + lcls)
    return np.asarray([loss, lbox, lobj, lcls], dtype=np.float32), res


def kernel(**inputs) -> np.ndarray:
    pred_full = [
        np.asarray(inputs[f"pred{l}"], dtype=np.float32) for l in range(3)
    ]
    targets = np.asarray(inputs["targets"], dtype=np.float32)
    out, _ = _run(pred_full, targets, trace=False)
    return out
